# revision 6
# baseline (speedup 1.0000x reference)
"""Trainium2 Bass kernel for the Chowder model (nn_Chowder_16080357556255) — fp8 TensorE pipeline.

Full-input contract: kernel(**inputs) takes the complete unsharded arrays and
returns the full [8, 1, 2] output.

Strategy (data-parallel over batch, per the sharding hint):
  - 8 NeuronCores, core i gets batch row i.
  - The heavy part is scores[n] = dot(x_i[n, :], conv_w), n < 50000, L=512.
    Memory-bound, so the host stages x fp8_e4m3 (25.6 MB/core vs 102.4) and
    TRANSPOSED so the 512-long contraction lands on the partition axis:
    TensorE does the dot products (DoubleRow fp8, K=256 per matmul, M=1),
    accumulating each 512-wide window into a PSUM bank at partition 0.
  - 1 MB contiguous x tiles (2048 scores each), all input DMAs issued up
    front on the Sync queue (prefetch gated only by buffer recycling);
    ScalarE/VectorE alternate draining [1, 1024] PSUM quarters into bf16
    SBUF; per-tile output DMAs ride the ACT queue so they never block input
    prefetch.
  - Host: top/bottom-64 candidates per bag from fp8 scores, recompute those
    rows in f64 from the original f32 x, exact top-5/bottom-5 + bias + MLP.
    End-to-end error is f32-roundoff (~3e-7), independent of fp8 staging.
"""

import os
import sys

for _p in ("/opt/trn_rl_repo",):
    if os.path.isdir(_p) and _p not in sys.path:
        sys.path.insert(0, _p)

import ml_dtypes
import numpy as np

import concourse.bass as bass  # noqa: E402
import concourse.tile as tile  # noqa: E402
from concourse import bacc, mybir  # noqa: E402
from concourse.bass_utils import run_bass_kernel_spmd  # noqa: E402

# Problem shapes (hardcoded per contract)
B, N, L, R, C = 8, 50000, 512, 5, 2
P = 128               # SBUF partitions = PE contraction size
QD = L // P           # 4 k-chunks of 128
F = 2048              # score columns per DMA tile (1 MB per tile)
NTF = N // F          # 24 full tiles
FT = 1024             # tail tile columns (covers 50000-49152=848, zero pad)
NPAD = NTF * F + FT   # 50176
WIN = 512             # matmul moving free dim / PSUM bank (f32)
QTR = 1024            # psum tile free size (2 banks, 2 windows)
WSCALE = 16.0         # conv_w prescale so fp8 hits normal range (exact pow2)
KCAND = 64            # host refinement candidate count per side

TAIL_FIRST = True     # process the small tail tile first (earliest data)
N_WARM = 64           # PE warm-up matmuls (HAM un-throttle before the stream)
FINE_TAIL = True      # per-window drains on the last tile (shorter tail)

F32 = mybir.dt.float32
BF16 = mybir.dt.bfloat16
F8 = mybir.dt.float8e4
NP_F8 = ml_dtypes.float8_e4m3
DR = mybir.MatmulPerfMode.DoubleRow


def build_nc(x_bufs: int = 16):
    """Per-core Bass program: scores[n] = sum_l xT8[l, n] * w8[l] via TensorE."""
    nc = bacc.Bacc(
        "TRN2", target_bir_lowering=False, debug=False, num_devices=B
    )
    # xt[t, p, q, n] = fp8(x[t*F + n, q*128 + p]); contiguous per tile.
    xt = nc.dram_tensor("xt", [NTF, P, QD, F], F8, kind="ExternalInput").ap()
    # tail tile, columns 49152..50175 (zero-padded past 50000)
    xtl = nc.dram_tensor("xtl", [P, QD, FT], F8, kind="ExternalInput").ap()
    # wt[p, j, c] = fp8(WSCALE * conv_w[256*c + 128*j + p]) for c < 2
    wt = nc.dram_tensor("wt", [P, 2, 32], F8, kind="ExternalInput").ap()
    out = nc.dram_tensor("scores", [NPAD], BF16, kind="ExternalOutput").ap()

    NT = NTF + 1
    with tile.TileContext(nc) as tc:
        with (
            tc.tile_pool(name="const", bufs=1) as const_pool,
            tc.tile_pool(name="x", bufs=x_bufs) as xpool,
            tc.tile_pool(name="s", bufs=4) as spool,
            tc.psum_pool(name="ps", bufs=4) as pspool,
        ):
            w_tile = const_pool.tile([P, 2, 32], F8)
            nc.sync.dma_start(out=w_tile[:], in_=wt)

            # Tail tile (0.5 MB) first: its data lands soonest, so the PE
            # stream starts ~1us earlier.
            order = ([NT - 1] + list(range(NT - 1))) if TAIL_FIRST \
                else list(range(NT))

            # All input DMAs issued up front on the Sync queue so prefetch is
            # gated only by x-buffer recycling, never by the per-tile
            # drain/store chain (output DMAs go on the ACT queue instead).
            xtiles = {}
            for t in order:
                x_t = xpool.tile([P, QD, F], F8, tag="xt")
                nc.sync.dma_start(
                    out=x_t[:, :, 0:FT if t == NT - 1 else F],
                    in_=xtl if t == NT - 1 else xt[t],
                )
                xtiles[t] = x_t

            # HAM warm-up: ~4.5us of tiny matmuls on the weight tile while
            # the first x tile is still in flight, so the real MM stream
            # starts at the full 2.4 GHz clock instead of 1.2.
            if N_WARM:
                # borrows one rotation slot of the main psum pool; free again
                # long before the 4th real quarter needs it
                warm_ps = pspool.tile([1, QTR], F32, tag="ps")
                for i in range(N_WARM):
                    nc.tensor.matmul(
                        out=warm_ps[:, 0:32],
                        lhsT=w_tile[:, :, 0:1],
                        rhs=w_tile[:],
                        start=True,
                        stop=True,
                        perf_mode=DR,
                    )

            drain = [nc.scalar.copy, nc.vector.tensor_copy]
            for t in order:
                fcols = FT if t == NT - 1 else F
                nwin = fcols // WIN        # 4 full, 2 tail
                x_t = xtiles[t]
                s_t = spool.tile([1, F], BF16, tag="st")
                # finer drains on the last processed tile shorten the tail
                qtr = WIN if (FINE_TAIL and t == order[-1]) else QTR
                for q in range(max(nwin * WIN // qtr, 1)):
                    qw = min(nwin, qtr // WIN)    # windows in this quarter
                    qd = qw * WIN
                    ps = pspool.tile([1, QTR], F32, tag="ps")
                    for w in range(qw):
                        n0 = q * qtr + w * WIN
                        for c in range(QD // 2):
                            nc.tensor.matmul(
                                out=ps[:, w * WIN:(w + 1) * WIN],
                                lhsT=w_tile[:, :, c:c + 1],
                                rhs=x_t[:, 2 * c:2 * c + 2, n0:n0 + WIN],
                                start=(c == 0),
                                stop=(c == QD // 2 - 1),
                                perf_mode=DR,
                            )
                    drain[(2 * t + q) % 2](
                        out=s_t[:, q * qtr:q * qtr + qd],
                        in_=ps[:, 0:qd],
                    )
                nc.scalar.dma_start(
                    out=out[t * F:t * F + fcols].rearrange(
                        "(a f) -> a f", a=1
                    ),
                    in_=s_t[:, 0:fcols],
                )
    nc.compile()
    return nc


_NC_CACHE = {}


def _get_nc():
    if "nc" not in _NC_CACHE:
        _NC_CACHE["nc"] = build_nc()
    return _NC_CACHE["nc"]


def stage_inputs(x, conv_w):
    """Host staging: transposed, tiled, fp8 x; prescaled fp8 conv_w layout."""
    x = np.asarray(x, dtype=np.float32)
    conv_w = np.asarray(conv_w, dtype=np.float32)
    # l = q*128 + p
    xq = x.reshape(B, N, QD, P)
    nfull = NTF * F
    # full tiles: [b, n, q, p] -> [b, t, p, q, n]
    xs8 = np.ascontiguousarray(
        xq[:, :nfull]
        .reshape(B, NTF, F, QD, P)
        .transpose(0, 1, 4, 3, 2)
    ).astype(NP_F8)
    xtl8 = np.zeros((B, P, QD, FT), dtype=NP_F8)
    xtl8[:, :, :, : N - nfull] = (
        xq[:, nfull:].transpose(0, 3, 2, 1).astype(NP_F8)
    )
    # wt[p, j, c] = w8[256c + 128j + p] (c < 2); j-stride 32 keeps the
    # DoubleRow weight AP step a multiple of 16
    wlq = (conv_w * WSCALE).astype(NP_F8).reshape(QD, P)  # [q, p]
    wt = np.zeros((P, 2, 32), dtype=NP_F8)
    for c in range(QD // 2):
        for j in range(2):
            wt[:, j, c] = wlq[2 * c + j]
    return xs8, xtl8, wt


def _postprocess(scores_dev, x, conv_w, conv_b, w1, b1, w2, b2, w3, b3):
    """Host tail: candidate top/bottom-K from fp8 scores, exact refinement,
    bias, tiny MLP."""
    x = np.asarray(x, dtype=np.float32)
    w64 = np.asarray(conv_w, dtype=np.float64)
    outs = []
    for b in range(B):
        s = scores_dev[b]
        ch = np.argpartition(s, N - KCAND)[N - KCAND:]
        cl = np.argpartition(s, KCAND - 1)[:KCAND]
        sh = x[b, ch].astype(np.float64) @ w64
        sl = x[b, cl].astype(np.float64) @ w64
        hi = np.sort(sh)[-R:][::-1]
        lo = np.sort(sl)[:R]
        cat = np.concatenate([lo, hi]) + np.float64(conv_b[0])
        h = cat @ np.asarray(w1, np.float64) + np.asarray(b1, np.float64)
        h = h @ np.asarray(w2, np.float64) + np.asarray(b2, np.float64)
        o = h @ np.asarray(w3, np.float64) + np.asarray(b3, np.float64)
        outs.append(o)
    return np.stack(outs)[:, None, :].astype(np.float32)  # [B, 1, C]


def kernel(
    x, conv_w, conv_b, w1, b1, w2, b2, w3, b3, _trace=False, _trace_kwargs=None
):
    xs8, xtl8, wt = stage_inputs(x, conv_w)
    nc = _get_nc()
    in_maps = [{"xt": xs8[i], "xtl": xtl8[i], "wt": wt} for i in range(B)]
    res = run_bass_kernel_spmd(
        nc,
        in_maps,
        list(range(B)),
        trace=_trace,
        **(_trace_kwargs or {}),
    )
    scores = np.stack(
        [res.results[i]["scores"][:N].astype(np.float64) for i in range(B)]
    ) / WSCALE  # [B, N]
    out = _postprocess(scores, x, conv_w, conv_b, w1, b1, w2, b2, w3, b3)
    if _trace:
        return out, res
    return out


# revision 7
# speedup vs baseline: 1.1792x; 1.1792x over previous
"""Trainium2 Bass kernel for the Chowder model (nn_Chowder_16080357556255) — fp8 TensorE pipeline.

Full-input contract: kernel(**inputs) takes the complete unsharded arrays and
returns the full [8, 1, 2] output.

Strategy (data-parallel over batch, per the sharding hint):
  - 8 NeuronCores, core i gets batch row i.
  - The heavy part is scores[n] = dot(x_i[n, :], conv_w), n < 50000, L=512.
    Memory-bound, so the host stages x fp8_e4m3 (25.6 MB/core vs 102.4) and
    TRANSPOSED so the 512-long contraction lands on the partition axis:
    TensorE does the dot products (DoubleRow fp8, K=256 per matmul, M=1),
    accumulating each 512-wide window into a PSUM bank at partition 0.
  - 1 MB contiguous x tiles (2048 scores each), all input DMAs issued up
    front on the Sync queue (prefetch gated only by buffer recycling);
    ScalarE/VectorE alternate draining [1, 1024] PSUM quarters into bf16
    SBUF; per-tile output DMAs ride the ACT queue so they never block input
    prefetch.
  - Host: top/bottom-64 candidates per bag from fp8 scores, recompute those
    rows in f64 from the original f32 x, exact top-5/bottom-5 + bias + MLP.
    End-to-end error is f32-roundoff (~3e-7), independent of fp8 staging.
"""

import os
import sys

for _p in ("/opt/trn_rl_repo",):
    if os.path.isdir(_p) and _p not in sys.path:
        sys.path.insert(0, _p)

import ml_dtypes
import numpy as np

import concourse.bass as bass  # noqa: E402
import concourse.tile as tile  # noqa: E402
from concourse import bacc, mybir  # noqa: E402
from concourse.bass_utils import run_bass_kernel_spmd  # noqa: E402

# Problem shapes (hardcoded per contract)
B, N, L, R, C = 8, 50000, 512, 5, 2
P = 128               # SBUF partitions = PE contraction size
QD = L // P           # 4 k-chunks of 128
F = 2048              # score columns per DMA tile (1 MB per tile)
NTF = N // F          # 24 full tiles
FT = 1024             # tail tile columns (covers 50000-49152=848, zero pad)
NPAD = NTF * F + FT   # 50176
WIN = 512             # matmul moving free dim / PSUM bank (f32)
QTR = 1024            # psum tile free size (2 banks, 2 windows)
WSCALE = 16.0         # conv_w prescale so fp8 hits normal range (exact pow2)
KCAND = 64            # host refinement candidate count per side

TAIL_FIRST = True     # process the small tail tile first (earliest data)
N_WARM = 64           # PE warm-up matmuls (HAM un-throttle before the stream)
FINE_TAIL = True      # per-window drains on the last tile (shorter tail)

F32 = mybir.dt.float32
BF16 = mybir.dt.bfloat16
F8 = mybir.dt.float8e4
NP_F8 = ml_dtypes.float8_e4m3
DR = mybir.MatmulPerfMode.DoubleRow


def build_nc(x_bufs: int = 12):
    """Per-core Bass program: scores[n] = sum_l xT8[l, n] * w8[l] via TensorE."""
    nc = bacc.Bacc(
        "TRN2", target_bir_lowering=False, debug=False, num_devices=B
    )
    # xt[t, p, q, n] = fp8(x[t*F + n, q*128 + p]); contiguous per tile.
    xt = nc.dram_tensor("xt", [NTF, P, QD, F], F8, kind="ExternalInput").ap()
    # tail tile, columns 49152..50175 (zero-padded past 50000)
    xtl = nc.dram_tensor("xtl", [P, QD, FT], F8, kind="ExternalInput").ap()
    # wt[p, j, c] = fp8(WSCALE * conv_w[256*c + 128*j + p]) for c < 2
    wt = nc.dram_tensor("wt", [P, 2, 32], F8, kind="ExternalInput").ap()
    out = nc.dram_tensor("scores", [NPAD], BF16, kind="ExternalOutput").ap()

    NT = NTF + 1
    with tile.TileContext(nc) as tc:
        with (
            tc.tile_pool(name="const", bufs=1) as const_pool,
            tc.tile_pool(name="x", bufs=x_bufs) as xpool,
            tc.tile_pool(name="s", bufs=4) as spool,
            tc.psum_pool(name="ps", bufs=4) as pspool,
        ):
            w_tile = const_pool.tile([P, 2, 32], F8)
            nc.sync.dma_start(out=w_tile[:], in_=wt)

            # Tail tile (0.5 MB) first: its data lands soonest, so the PE
            # stream starts ~1us earlier.
            order = ([NT - 1] + list(range(NT - 1))) if TAIL_FIRST \
                else list(range(NT))

            # All input DMAs issued up front on the Sync queue so prefetch is
            # gated only by x-buffer recycling, never by the per-tile
            # drain/store chain (output DMAs go on the ACT queue instead).
            xtiles = {}
            for t in order:
                x_t = xpool.tile([P, QD, F], F8, tag="xt")
                nc.sync.dma_start(
                    out=x_t[:, :, 0:FT if t == NT - 1 else F],
                    in_=xtl if t == NT - 1 else xt[t],
                )
                xtiles[t] = x_t

            # HAM warm-up: ~4.5us of tiny matmuls on the weight tile while
            # the first x tile is still in flight, so the real MM stream
            # starts at the full 2.4 GHz clock instead of 1.2.
            if N_WARM:
                # borrows one rotation slot of the main psum pool; free again
                # long before the 4th real quarter needs it
                warm_ps = pspool.tile([1, QTR], F32, tag="ps")
                for i in range(N_WARM):
                    nc.tensor.matmul(
                        out=warm_ps[:, 0:32],
                        lhsT=w_tile[:, :, 0:1],
                        rhs=w_tile[:],
                        start=True,
                        stop=True,
                        perf_mode=DR,
                    )

            drain = [nc.scalar.copy, nc.vector.tensor_copy]
            for t in order:
                fcols = FT if t == NT - 1 else F
                nwin = fcols // WIN        # 4 full, 2 tail
                x_t = xtiles[t]
                s_t = spool.tile([1, F], BF16, tag="st")
                # finer drains on the last processed tile shorten the tail
                qtr = WIN if (FINE_TAIL and t == order[-1]) else QTR
                for q in range(max(nwin * WIN // qtr, 1)):
                    qw = min(nwin, qtr // WIN)    # windows in this quarter
                    qd = qw * WIN
                    ps = pspool.tile([1, QTR], F32, tag="ps")
                    for w in range(qw):
                        n0 = q * qtr + w * WIN
                        for c in range(QD // 2):
                            nc.tensor.matmul(
                                out=ps[:, w * WIN:(w + 1) * WIN],
                                lhsT=w_tile[:, :, c:c + 1],
                                rhs=x_t[:, 2 * c:2 * c + 2, n0:n0 + WIN],
                                start=(c == 0),
                                stop=(c == QD // 2 - 1),
                                perf_mode=DR,
                            )
                    drain[(2 * t + q) % 2](
                        out=s_t[:, q * qtr:q * qtr + qd],
                        in_=ps[:, 0:qd],
                    )
                nc.scalar.dma_start(
                    out=out[t * F:t * F + fcols].rearrange(
                        "(a f) -> a f", a=1
                    ),
                    in_=s_t[:, 0:fcols],
                )
    nc.compile()
    return nc


_NC_CACHE = {}


def _get_nc():
    if "nc" not in _NC_CACHE:
        _NC_CACHE["nc"] = build_nc()
    return _NC_CACHE["nc"]


def stage_inputs(x, conv_w):
    """Host staging: transposed, tiled, fp8 x; prescaled fp8 conv_w layout."""
    x = np.asarray(x, dtype=np.float32)
    conv_w = np.asarray(conv_w, dtype=np.float32)
    # l = q*128 + p
    xq = x.reshape(B, N, QD, P)
    nfull = NTF * F
    # full tiles: [b, n, q, p] -> [b, t, p, q, n]
    xs8 = np.ascontiguousarray(
        xq[:, :nfull]
        .reshape(B, NTF, F, QD, P)
        .transpose(0, 1, 4, 3, 2)
    ).astype(NP_F8)
    xtl8 = np.zeros((B, P, QD, FT), dtype=NP_F8)
    xtl8[:, :, :, : N - nfull] = (
        xq[:, nfull:].transpose(0, 3, 2, 1).astype(NP_F8)
    )
    # wt[p, j, c] = w8[256c + 128j + p] (c < 2); j-stride 32 keeps the
    # DoubleRow weight AP step a multiple of 16
    wlq = (conv_w * WSCALE).astype(NP_F8).reshape(QD, P)  # [q, p]
    wt = np.zeros((P, 2, 32), dtype=NP_F8)
    for c in range(QD // 2):
        for j in range(2):
            wt[:, j, c] = wlq[2 * c + j]
    return xs8, xtl8, wt


def _postprocess(scores_dev, x, conv_w, conv_b, w1, b1, w2, b2, w3, b3):
    """Host tail: candidate top/bottom-K from fp8 scores, exact refinement,
    bias, tiny MLP."""
    x = np.asarray(x, dtype=np.float32)
    w64 = np.asarray(conv_w, dtype=np.float64)
    outs = []
    for b in range(B):
        s = scores_dev[b]
        ch = np.argpartition(s, N - KCAND)[N - KCAND:]
        cl = np.argpartition(s, KCAND - 1)[:KCAND]
        sh = x[b, ch].astype(np.float64) @ w64
        sl = x[b, cl].astype(np.float64) @ w64
        hi = np.sort(sh)[-R:][::-1]
        lo = np.sort(sl)[:R]
        cat = np.concatenate([lo, hi]) + np.float64(conv_b[0])
        h = cat @ np.asarray(w1, np.float64) + np.asarray(b1, np.float64)
        h = h @ np.asarray(w2, np.float64) + np.asarray(b2, np.float64)
        o = h @ np.asarray(w3, np.float64) + np.asarray(b3, np.float64)
        outs.append(o)
    return np.stack(outs)[:, None, :].astype(np.float32)  # [B, 1, C]


def kernel(
    x, conv_w, conv_b, w1, b1, w2, b2, w3, b3, _trace=False, _trace_kwargs=None
):
    xs8, xtl8, wt = stage_inputs(x, conv_w)
    nc = _get_nc()
    in_maps = [{"xt": xs8[i], "xtl": xtl8[i], "wt": wt} for i in range(B)]
    res = run_bass_kernel_spmd(
        nc,
        in_maps,
        list(range(B)),
        trace=_trace,
        **(_trace_kwargs or {}),
    )
    scores = np.stack(
        [res.results[i]["scores"][:N].astype(np.float64) for i in range(B)]
    ) / WSCALE  # [B, N]
    out = _postprocess(scores, x, conv_w, conv_b, w1, b1, w2, b2, w3, b3)
    if _trace:
        return out, res
    return out
